# revision 16
# baseline (speedup 1.0000x reference)
"""Trainium2 Bass kernel for nn_CRModule (retrieval_knn).

reference:
    xf = x.reshape(4096, 4096); xa = xf[:, ::2]; xb = xf[:, 1::2]   # [T=4096, 2048]
    sq[i,j] = |xa[:,i]|^2 + |xb[:,j]|^2 - 2 * xa[:,i].xb[:,j]
    wsum = fc_weight.sum(0); wa = wsum[::2]; wb = wsum[1::2]
    scores[i,j] = ((wa[i] + wb[j]) * sqrt(max(sq,0)))**2
                = (wa[i] + wb[j])**2 * max(sq[i,j], 0)     # sqrt cancels

Strategy (8 NeuronCores, two SPMD launches):
  Launch 1 (cross-core reductions, host combines 24 KB):
    fc_weight row-sharded (1536 rows/core, columns pre-split [even|odd])
    -> partial column sums wpart = [wa_part | wb_part]; xb column norms
    sharded over channels (256/core) -> nbsl.
  Launch 2 (main, row-sharded output): each core owns 256 rows of scores;
    (-2a)^T b in bf16 on PE, k-OUTER accumulation into all 8 PSUM banks
    so matmuls chase the chunked xb DMA stream; na computed on-device
    from (-2a)^2 * 0.25; fused fp32 DVE epilogue in-place in PSUM.
"""

import numpy as np
import ml_dtypes

import concourse.bass as bass
import concourse.tile as tile
from concourse import bacc, mybir
from concourse.bass_utils import run_bass_kernel_spmd

BF16 = mybir.dt.bfloat16
F32 = mybir.dt.float32
NP_BF16 = ml_dtypes.bfloat16
FP8 = mybir.dt.float8e4
NP_FP8 = ml_dtypes.float8_e4m3

D = 8          # cores
T = 4096       # inner (contraction) dim = B*N
KT = T // 128  # 32 k-tiles
CA = 2048      # C/2 channels
MLOC = CA // D  # 256 output rows per core
O = 12288      # fc rows
OLOC = O // D   # 1536 fc rows per core
OT = OLOC // 128  # 12 o-tiles per core
C = 4096

_cache = {}


def _new_nc():
    return bacc.Bacc("TRN2", target_bir_lowering=False, debug=False, num_devices=D)


def _build_phase1():
    """Per-core: partial fc column-sum (cols pre-split [even|odd]) +
    sharded xb column sq-norms."""
    nc = _new_nc()
    fc_d = nc.dram_tensor("fc", [128, OT, C], BF16, kind="ExternalInput").ap()
    xbs_d = nc.dram_tensor("xbs", [128, KT, MLOC], BF16, kind="ExternalInput").ap()
    wpart_d = nc.dram_tensor("wpart", [1, C], F32, kind="ExternalOutput").ap()
    nbsl_d = nc.dram_tensor("nbsl", [1, MLOC], F32, kind="ExternalOutput").ap()

    with tile.TileContext(nc) as tc:
        with (
            tc.tile_pool(name="fcp", bufs=1) as fcp,
            tc.tile_pool(name="xp", bufs=1) as xp,
            tc.tile_pool(name="small", bufs=1) as small,
            tc.tile_pool(name="stage", bufs=2) as stage,
            tc.tile_pool(name="psw", bufs=4, space="PSUM") as psw,
            tc.tile_pool(name="psn", bufs=1, space="PSUM") as psn,
        ):
            ones = small.tile([128, 1], BF16)
            nc.vector.memset(ones[:], 1.0)

            # xb slice first (small), then chunked fc load
            xbs_sb = xp.tile([128, KT, MLOC], BF16)
            nc.sync.dma_start(xbs_sb[:], xbs_d[:])
            ft = []
            for ot in range(OT):
                f = fcp.tile([128, C], BF16, tag=f"fc{ot}")
                nc.sync.dma_start(f[:], fc_d[:, ot, :])
                ft.append(f)

            # nb slice: square on ScalarE, column-sum over 32 k-tiles
            x2 = xp.tile([128, KT, MLOC], BF16)
            nc.scalar.square(x2[:], xbs_sb[:])
            psb = psn.tile([1, MLOC], F32)
            for kt in range(KT):
                nc.tensor.matmul(
                    psb[:], ones[:], x2[:, kt, :],
                    start=(kt == 0), stop=(kt == KT - 1),
                )
            st = stage.tile([1, MLOC], F32)
            nc.vector.tensor_copy(st[:], psb[:])
            nc.sync.dma_start(nbsl_d[:], st[:])

            # partial fc column sums; 4 psum banks per half
            wsb = stage.tile([1, C], F32)
            for half in range(2):
                pss = [psw.tile([1, 512], F32, name=f"psw{half}_{i}", tag="psw")
                       for i in range(4)]
                for ot in range(OT):
                    for ci, ps in enumerate(pss):
                        ch = half * 4 + ci
                        nc.tensor.matmul(
                            ps[:], ones[:],
                            ft[ot][:, ch * 512:(ch + 1) * 512],
                            start=(ot == 0), stop=(ot == OT - 1),
                        )
                for ci, ps in enumerate(pss):
                    ch = half * 4 + ci
                    nc.vector.tensor_copy(wsb[:, ch * 512:(ch + 1) * 512], ps[:])
            nc.sync.dma_start(wpart_d[:], wsb[:])

    nc.compile()
    return nc


def _build_phase2():
    """Per-core: 256 rows of scores = (wa+wb)^2 * relu(na+nb-2ab)."""
    nc = _new_nc()
    xasc_d = nc.dram_tensor("xasc", [128, KT, MLOC], FP8, kind="ExternalInput").ap()
    xbr_d = nc.dram_tensor("xbr", [128, KT, CA], FP8, kind="ExternalInput").ap()
    # wa per-partition per m-tile
    pv_d = nc.dram_tensor("pv", [128, 2], F32, kind="ExternalInput").ap()
    # packed free-axis vectors: [0, 0:CA]=wb, [0, CA:2CA]=nb
    fv_d = nc.dram_tensor("fv", [1, 2 * CA], F32, kind="ExternalInput").ap()
    out_d = nc.dram_tensor("scores", [MLOC, CA], F32, kind="ExternalOutput").ap()
    na_dram = nc.dram_tensor("na_tmp", [1, MLOC], F32).ap()

    NJ = CA // 512   # 4 column chunks
    MT = MLOC // 128  # 2 m-tiles
    KG = 2           # k-tiles per xb DMA chunk
    XG = 8           # k-tiles per xa DMA chunk

    with tile.TileContext(nc) as tc:
        with (
            tc.tile_pool(name="xap", bufs=1) as xap,
            tc.tile_pool(name="xbp", bufs=1) as xbp,
            tc.tile_pool(name="small", bufs=1) as small,
            tc.tile_pool(name="w2p", bufs=1) as w2p,
            tc.tile_pool(name="x2p", bufs=2) as x2p,
            tc.tile_pool(name="outp", bufs=2) as outp,
        ):
            # ---- input streams (emission order = DMA issue order) ----
            xac = []
            for g in range(KT // XG):
                x_t = xap.tile([128, XG, MLOC], FP8, tag=f"xa{g}")
                nc.sync.dma_start(x_t[:], xasc_d[:, g * XG:(g + 1) * XG, :])
                xac.append(x_t)
            xbt = []
            for h in range(KT // KG):
                xb_t = xbp.tile([128, KG, CA], FP8, tag=f"xb{h}")
                nc.sync.dma_start(xb_t[:], xbr_d[:, h * KG:(h + 1) * KG, :])
                xbt.append(xb_t)

            quarter = small.tile([128, 1], BF16)
            nc.vector.memset(quarter[:], 0.25)

            # ---- na from (-2a)^2 * 0.25, then DRAM roundtrip to [128,2] ----
            with tc.tile_pool(name="psna", bufs=1, space="PSUM") as psna:
                psa = psna.tile([1, MLOC], F32)
                for g in range(KT // XG):
                    x2 = x2p.tile([128, XG, MLOC], BF16, tag="x2")
                    nc.scalar.square(x2[:], xac[g][:])
                    for s in range(XG):
                        kt = g * XG + s
                        nc.tensor.matmul(
                            psa[:], quarter[:], x2[:, s, :],
                            start=(kt == 0), stop=(kt == KT - 1),
                        )
                nast = small.tile([1, MLOC], F32)
                nc.vector.tensor_copy(nast[:], psa[:])
                nc.sync.dma_start(na_dram[:], nast[:])
            nav = small.tile([128, MT], F32)
            nc.sync.dma_start(
                nav[:],
                bass.AP(tensor=na_dram.tensor, offset=0, ap=[[1, 128], [128, MT]]),
            )

            # ---- main matmul: k-OUTER accumulation, 2 x 4-bank psum tiles ----
            with tc.tile_pool(name="psmm", bufs=2, space="PSUM") as psmm:
                ps = [psmm.tile([128, NJ, 512], F32, name=f"ps{m}", tag="ps")
                      for m in range(MT)]
                for kt in range(KT):
                    h, r = divmod(kt, KG)
                    g, s = divmod(kt, XG)
                    for m in range(MT):
                        for nj in range(NJ):
                            nc.tensor.matmul(
                                ps[m][:, nj, :],
                                xac[g][:, s, m * 128:(m + 1) * 128],
                                xbt[h][:, r, nj * 512:(nj + 1) * 512],
                                start=(kt == 0), stop=(kt == KT - 1),
                            )

                # ---- epilogue vectors (issued late; DMA overlaps MM stream) ----
                pv = small.tile([128, 2], F32)
                nc.sync.dma_start(pv[:], pv_d[:])
                wb_bc = small.tile([128, CA], F32)
                nc.sync.dma_start(wb_bc[:], fv_d[0:1, 0:CA].to_broadcast([128, CA]))
                nb_bc = small.tile([128, CA], F32)
                nc.sync.dma_start(nb_bc[:], fv_d[0:1, CA:2 * CA].to_broadcast([128, CA]))
                w2 = []
                for m in range(MT):
                    w2m = w2p.tile([128, CA], F32, tag=f"w2_{m}")
                    nc.scalar.activation(
                        w2m[:], wb_bc[:],
                        mybir.ActivationFunctionType.Square,
                        bias=pv[:, m:m + 1], scale=1.0,
                    )
                    w2.append(w2m)

                # ---- epilogue: sq in-place in psum, scale, store ----
                for m in range(MT):
                    pflat = ps[m].rearrange("p a b -> p (a b)")
                    nc.vector.scalar_tensor_tensor(
                        pflat, pflat, nav[:, m:m + 1], nb_bc[:],
                        op0=mybir.AluOpType.add, op1=mybir.AluOpType.add,
                    )
                    ot = outp.tile([128, CA], F32, tag="ot")
                    nc.vector.scalar_tensor_tensor(
                        ot[:], pflat, 0.0, w2[m][:],
                        op0=mybir.AluOpType.max, op1=mybir.AluOpType.mult,
                    )
                    nc.sync.dma_start(out_d[m * 128:(m + 1) * 128, :], ot[:])

    nc.compile()
    return nc


def _p_major(a, np_dtype):
    """[n*128, cols] -> [128, n, cols] with tile index in the middle."""
    n = a.shape[0] // 128
    return np.ascontiguousarray(
        a.reshape(n, 128, a.shape[1]).transpose(1, 0, 2).astype(np_dtype)
    )


def _kernel_twolaunch(x, fc_weight, _trace=False):
    x = np.asarray(x, dtype=np.float32)
    fc = np.asarray(fc_weight, dtype=np.float32)

    xf = x.reshape(T, C)
    xa = np.ascontiguousarray(xf[:, 0::2])   # [T, CA]
    xb = np.ascontiguousarray(xf[:, 1::2])
    # deinterleave fc columns: [even | odd] so wpart = [wa_part | wb_part]
    fc_r = np.concatenate([fc[:, 0::2], fc[:, 1::2]], axis=1)

    xb_r = _p_major(xb, NP_FP8)              # [128, KT, CA]
    xa_s2 = -2.0 * xa

    # ---- launch 1 ----
    if "p1" not in _cache:
        _cache["p1"] = _build_phase1()
    nc1 = _cache["p1"]

    in_maps1 = []
    for d in range(D):
        sl = slice(d * MLOC, (d + 1) * MLOC)
        in_maps1.append({
            "fc": _p_major(fc_r[d * OLOC:(d + 1) * OLOC], NP_BF16),
            "xbs": _p_major(xb[:, sl], NP_BF16),
        })
    res1 = run_bass_kernel_spmd(nc1, in_maps1, core_ids=list(range(D)), trace=_trace)
    t1 = res1.exec_time_ns

    wsum = np.sum([res1.results[d]["wpart"][0] for d in range(D)], axis=0,
                  dtype=np.float32)                              # [C] = [wa|wb]
    nb = np.concatenate([res1.results[d]["nbsl"][0] for d in range(D)])
    wa, wb = wsum[:CA], wsum[CA:]

    # ---- launch 2 ----
    if "p2" not in _cache:
        _cache["p2"] = _build_phase2()
    nc2 = _cache["p2"]

    fv = np.concatenate([wb, nb]).reshape(1, 2 * CA).astype(np.float32)
    in_maps2 = []
    for d in range(D):
        sl = slice(d * MLOC, (d + 1) * MLOC)
        in_maps2.append({
            "xasc": _p_major(xa_s2[:, sl], NP_FP8),
            "xbr": xb_r,
            "pv": np.ascontiguousarray(wa[sl].reshape(2, 128).T).astype(np.float32),
            "fv": fv,
        })
    res2 = run_bass_kernel_spmd(nc2, in_maps2, core_ids=list(range(D)), trace=_trace)
    t2 = res2.exec_time_ns

    out = np.concatenate([res2.results[d]["scores"] for d in range(D)], axis=0)
    if _trace:
        kernel.last_times = (t1, t2)
    return out.astype(np.float32)


def _build_merged():
    """Single launch, fc column-sharded per core, odd(wb)/even(wa) halves as
    separate streams: wb half loads first so its AllGather issues early.
    Main matmul fp8; fused fp32 epilogue in PSUM."""
    nc = _new_nc()
    OTT = O // 128    # 96 fc o-tiles
    FG = 16           # o-tiles per fc DMA chunk -> 6 chunks per half
    fcb_d = nc.dram_tensor("fcb", [128, OTT, MLOC], BF16, kind="ExternalInput").ap()
    fca_d = nc.dram_tensor("fca", [128, OTT, MLOC], BF16, kind="ExternalInput").ap()
    xasc_d = nc.dram_tensor("xasc", [128, KT, MLOC], FP8, kind="ExternalInput").ap()
    xbs_d = nc.dram_tensor("xbs", [128, KT, MLOC], FP8, kind="ExternalInput").ap()
    xbr_d = nc.dram_tensor("xbr", [128, KT, CA], FP8, kind="ExternalInput").ap()
    out_d = nc.dram_tensor("scores", [MLOC, CA], F32, kind="ExternalOutput").ap()

    nb_in = nc.dram_tensor("nb_in", [1, MLOC], F32).ap()
    nb_sh = nc.dram_tensor("nb_sh", [D, MLOC], F32, addr_space="Shared").ap()
    wb_in = nc.dram_tensor("wb_in", [1, MLOC], F32).ap()
    wb_sh = nc.dram_tensor("wb_sh", [D, MLOC], F32, addr_space="Shared").ap()
    grp = [list(range(D))]

    NJ = CA // 512
    MT = MLOC // 128
    KG = 2            # k-tiles per xb DMA chunk
    XG = 8            # k-tiles per xa DMA chunk

    import contextlib
    with tile.TileContext(nc) as tc:
        es = contextlib.ExitStack()
        with es, \
             tc.tile_pool(name="xap", bufs=1) as xap, \
             tc.tile_pool(name="xsp", bufs=1) as xsp, \
             tc.tile_pool(name="xbp", bufs=1) as xbp, \
             tc.tile_pool(name="fbp", bufs=2) as fbp, \
             tc.tile_pool(name="fap", bufs=2) as fap, \
             tc.tile_pool(name="small", bufs=1) as small, \
             tc.tile_pool(name="w2p", bufs=1) as w2p, \
             tc.tile_pool(name="x2p", bufs=2) as x2p, \
             tc.tile_pool(name="outp", bufs=2) as outp, \
             tc.tile_pool(name="psm0", bufs=1, space="PSUM") as psm0:
            pse = es.enter_context(tc.tile_pool(name="pse", bufs=1, space="PSUM"))

            # ---- DMA emission: xbs, xasc, fcb (wb half), fca, xbr ----
            xbs_sb = xsp.tile([128, KT, MLOC], FP8)
            nc.sync.dma_start(xbs_sb[:], xbs_d[:])
            xac = []
            for g in range(KT // XG):
                x_t = xap.tile([128, XG, MLOC], FP8, tag=f"xa{g}")
                nc.sync.dma_start(x_t[:], xasc_d[:, g * XG:(g + 1) * XG, :])
                xac.append(x_t)
            fbt = []
            for rnd in range(OTT // FG):
                f = fbp.tile([128, FG, MLOC], BF16, tag="fcb")
                nc.sync.dma_start(f[:], fcb_d[:, rnd * FG:(rnd + 1) * FG, :])
                fbt.append(f)
            xbt = []
            fat = []
            for bi in range(KT // KG):
                xb_t = xbp.tile([128, KG, CA], FP8, tag=f"xb{bi}")
                nc.sync.dma_start(xb_t[:], xbr_d[:, bi * KG:(bi + 1) * KG, :])
                xbt.append(xb_t)
                if bi >= 5 and bi % 2 == 1:  # bi in {5,7,9,11,13,15}
                    rnd = len(fat)
                    f = fap.tile([128, FG, MLOC], BF16, tag="fca")
                    nc.sync.dma_start(f[:], fca_d[:, rnd * FG:(rnd + 1) * FG, :])
                    fat.append(f)

            ones = small.tile([128, 1], BF16)
            nc.vector.memset(ones[:], 1.0)
            quarter = small.tile([128, 1], BF16)
            nc.vector.memset(quarter[:], 0.25)
            onef = small.tile([1, 1], F32)
            nc.vector.memset(onef[:], 1.0)

            # ---- nb slice (feeds earliest AllGather) ----
            psb = pse.tile([1, MLOC], F32, name="psb", tag="psb")
            for g in range(KT // XG):
                x2b = x2p.tile([128, XG, MLOC], BF16, tag="x2b")
                nc.scalar.square(x2b[:], xbs_sb[:, g * XG:(g + 1) * XG, :])
                for st_ in range(XG):
                    kt = g * XG + st_
                    nc.tensor.matmul(psb[:], ones[:], x2b[:, st_, :],
                                     start=(kt == 0), stop=(kt == KT - 1))
            nbst = small.tile([1, MLOC], F32)
            nc.vector.tensor_copy(nbst[:], psb[:])
            nc.gpsimd.dma_start(nb_in[:], nbst[:])
            nc.gpsimd.collective_compute(
                "AllGather", mybir.AluOpType.bypass, replica_groups=grp,
                ins=[nb_in[:]], outs=[nb_sh[:]])

            # ---- fcb (odd cols): wb_part -> AllGather ASAP ----
            pswb = pse.tile([1, MLOC], F32, name="pswb", tag="bchain")
            for rnd in range(OTT // FG):
                for o in range(FG):
                    ot = rnd * FG + o
                    nc.tensor.matmul(pswb[:], ones[:], fbt[rnd][:, o, :],
                                     start=(ot == 0), stop=(ot == OTT - 1))
            wbst = small.tile([1, MLOC], F32)
            nc.vector.tensor_copy(wbst[:], pswb[:])
            nc.gpsimd.dma_start(wb_in[:], wbst[:])
            nc.gpsimd.collective_compute(
                "AllGather", mybir.AluOpType.bypass, replica_groups=grp,
                ins=[wb_in[:]], outs=[wb_sh[:]])
            # broadcast reads on gpsimd (gated only on the AGs)
            nb_bc = small.tile([128, CA], F32)
            nc.gpsimd.dma_start(nb_bc[:], bass.AP(tensor=nb_sh.tensor, offset=0,
                                                  ap=[[0, 128], [1, CA]]))
            wb_bc = small.tile([128, CA], F32)
            nc.gpsimd.dma_start(wb_bc[:], bass.AP(tensor=wb_sh.tensor, offset=0,
                                                  ap=[[0, 128], [1, CA]]))

            # ---- na local + transpose to [128, MT] via K=1 matmuls ----
            psa = pse.tile([1, MLOC], F32, name="psa", tag="psa")
            for g in range(KT // XG):
                x2 = x2p.tile([128, XG, MLOC], BF16, tag="x2")
                nc.scalar.square(x2[:], xac[g][:])
                for st_ in range(XG):
                    kt = g * XG + st_
                    nc.tensor.matmul(psa[:], quarter[:], x2[:, st_, :],
                                     start=(kt == 0), stop=(kt == KT - 1))
            nast = small.tile([1, MLOC], F32)
            nc.vector.tensor_copy(nast[:], psa[:])
            pst_a = pse.tile([128, MT], F32, name="pst_a", tag="wchain")
            for m in range(MT):
                nc.tensor.matmul(pst_a[:, m:m + 1],
                                 nast[0:1, m * 128:(m + 1) * 128], onef[:],
                                 start=(m == 0), stop=(m == MT - 1),
                                 skip_group_check=True)
            nav = small.tile([128, MT], F32)
            nc.vector.tensor_copy(nav[:], pst_a[:])

            # ---- m0 matmuls (chase xbr stream) ----
            ps0 = psm0.tile([128, NJ, 512], F32, name="ps0", tag="ps")
            for kt in range(KT):
                g, s_ = divmod(kt, XG)
                h, r_ = divmod(kt, KG)
                for nj in range(NJ):
                    nc.tensor.matmul(
                        ps0[:, nj, :],
                        xac[g][:, s_, 0:128],
                        xbt[h][:, r_, nj * 512:(nj + 1) * 512],
                        start=(kt == 0), stop=(kt == KT - 1))

            # ---- fca (even cols): wa local ----
            pswa = pse.tile([1, MLOC], F32, name="pswa", tag="wchain")
            for rnd in range(OTT // FG):
                for o in range(FG):
                    ot = rnd * FG + o
                    nc.tensor.matmul(pswa[:], ones[:], fat[rnd][:, o, :],
                                     start=(ot == 0), stop=(ot == OTT - 1))
            wast = small.tile([1, MLOC], F32)
            nc.vector.tensor_copy(wast[:], pswa[:])
            pst_w = pse.tile([128, MT], F32, name="pst_w", tag="wchain")
            for m in range(MT):
                nc.tensor.matmul(pst_w[:, m:m + 1],
                                 wast[0:1, m * 128:(m + 1) * 128], onef[:],
                                 start=(m == 0), stop=(m == MT - 1),
                                 skip_group_check=True)
            wav = small.tile([128, MT], F32)
            nc.vector.tensor_copy(wav[:], pst_w[:])

            es.close()

            w2 = []
            for m in range(MT):
                w2m = w2p.tile([128, CA], F32, tag=f"w2_{m}")
                nc.scalar.activation(w2m[:], wb_bc[:],
                                     mybir.ActivationFunctionType.Square,
                                     bias=wav[:, m:m + 1], scale=1.0)
                w2.append(w2m)

            with tc.tile_pool(name="psm1", bufs=1, space="PSUM") as psm1:
                ps1 = psm1.tile([128, NJ, 512], F32, name="ps1", tag="ps")
                for kt in range(KT):
                    g, s_ = divmod(kt, XG)
                    h, r_ = divmod(kt, KG)
                    for nj in range(NJ):
                        nc.tensor.matmul(
                            ps1[:, nj, :],
                            xac[g][:, s_, 128:256],
                            xbt[h][:, r_, nj * 512:(nj + 1) * 512],
                            start=(kt == 0), stop=(kt == KT - 1))

                for m, psm in ((0, ps0), (1, ps1)):
                    pflat = psm.rearrange("p a b -> p (a b)")
                    nc.vector.scalar_tensor_tensor(
                        pflat, pflat, nav[:, m:m + 1], nb_bc[:],
                        op0=mybir.AluOpType.add, op1=mybir.AluOpType.add)
                    ot = outp.tile([128, CA], F32, tag="ot")
                    nc.vector.scalar_tensor_tensor(
                        ot[:], pflat, 0.0, w2[m][:],
                        op0=mybir.AluOpType.max, op1=mybir.AluOpType.mult)
                    nc.sync.dma_start(out_d[m * 128:(m + 1) * 128, :], ot[:])

    nc.compile()
    return nc


def kernel_merged(x, fc_weight, _trace=False):
    x = np.asarray(x, dtype=np.float32)
    fc = np.asarray(fc_weight, dtype=np.float32)
    xf = x.reshape(T, C)
    xa = np.ascontiguousarray(xf[:, 0::2])
    xb = np.ascontiguousarray(xf[:, 1::2])
    xb_r = _p_major(xb, NP_FP8)
    xa_s2 = -2.0 * xa

    if "pm" not in _cache:
        _cache["pm"] = _build_merged()
    ncm = _cache["pm"]
    in_maps = []
    for d in range(D):
        sl = slice(d * MLOC, (d + 1) * MLOC)
        lo, hi = 2 * d * MLOC, 2 * (d + 1) * MLOC
        in_maps.append({
            "fcb": _p_major(np.ascontiguousarray(fc[:, lo + 1:hi:2]), NP_BF16),
            "fca": _p_major(np.ascontiguousarray(fc[:, lo:hi:2]), NP_BF16),
            "xasc": _p_major(xa_s2[:, sl], NP_FP8),
            "xbs": _p_major(xb[:, sl], NP_FP8),
            "xbr": xb_r,
        })
    res = run_bass_kernel_spmd(ncm, in_maps, core_ids=list(range(D)), trace=_trace)
    out = np.concatenate([res.results[d]["scores"] for d in range(D)], axis=0)
    if _trace:
        kernel_merged.last_times = (res.exec_time_ns,)
    return out.astype(np.float32)


def kernel(x, fc_weight):
    """Graded entrypoint: full inputs in, full [2048, 2048] scores out."""
    return kernel_merged(x, fc_weight)


# revision 17
# speedup vs baseline: 1.1131x; 1.1131x over previous
"""Trainium2 Bass kernel for nn_CRModule (retrieval_knn).

reference:
    xf = x.reshape(4096, 4096); xa = xf[:, ::2]; xb = xf[:, 1::2]   # [T=4096, 2048]
    sq[i,j] = |xa[:,i]|^2 + |xb[:,j]|^2 - 2 * xa[:,i].xb[:,j]
    wsum = fc_weight.sum(0); wa = wsum[::2]; wb = wsum[1::2]
    scores[i,j] = ((wa[i] + wb[j]) * sqrt(max(sq,0)))**2
                = (wa[i] + wb[j])**2 * max(sq[i,j], 0)     # sqrt cancels

Strategy (8 NeuronCores, two SPMD launches):
  Launch 1 (cross-core reductions, host combines 24 KB):
    fc_weight row-sharded (1536 rows/core, columns pre-split [even|odd])
    -> partial column sums wpart = [wa_part | wb_part]; xb column norms
    sharded over channels (256/core) -> nbsl.
  Launch 2 (main, row-sharded output): each core owns 256 rows of scores;
    (-2a)^T b in bf16 on PE, k-OUTER accumulation into all 8 PSUM banks
    so matmuls chase the chunked xb DMA stream; na computed on-device
    from (-2a)^2 * 0.25; fused fp32 DVE epilogue in-place in PSUM.
"""

import numpy as np
import ml_dtypes

import concourse.bass as bass
import concourse.tile as tile
from concourse import bacc, mybir
from concourse.bass_utils import run_bass_kernel_spmd

BF16 = mybir.dt.bfloat16
F32 = mybir.dt.float32
NP_BF16 = ml_dtypes.bfloat16
FP8 = mybir.dt.float8e4
NP_FP8 = ml_dtypes.float8_e4m3

D = 8          # cores
T = 4096       # inner (contraction) dim = B*N
KT = T // 128  # 32 k-tiles
CA = 2048      # C/2 channels
MLOC = CA // D  # 256 output rows per core
O = 12288      # fc rows
OLOC = O // D   # 1536 fc rows per core
OT = OLOC // 128  # 12 o-tiles per core
C = 4096

_cache = {}


def _new_nc():
    return bacc.Bacc("TRN2", target_bir_lowering=False, debug=False, num_devices=D)


def _build_phase1():
    """Per-core: partial fc column-sum (cols pre-split [even|odd]) +
    sharded xb column sq-norms."""
    nc = _new_nc()
    fc_d = nc.dram_tensor("fc", [128, OT, C], BF16, kind="ExternalInput").ap()
    xbs_d = nc.dram_tensor("xbs", [128, KT, MLOC], BF16, kind="ExternalInput").ap()
    wpart_d = nc.dram_tensor("wpart", [1, C], F32, kind="ExternalOutput").ap()
    nbsl_d = nc.dram_tensor("nbsl", [1, MLOC], F32, kind="ExternalOutput").ap()

    with tile.TileContext(nc) as tc:
        with (
            tc.tile_pool(name="fcp", bufs=1) as fcp,
            tc.tile_pool(name="xp", bufs=1) as xp,
            tc.tile_pool(name="small", bufs=1) as small,
            tc.tile_pool(name="stage", bufs=2) as stage,
            tc.tile_pool(name="psw", bufs=4, space="PSUM") as psw,
            tc.tile_pool(name="psn", bufs=1, space="PSUM") as psn,
        ):
            ones = small.tile([128, 1], BF16)
            nc.vector.memset(ones[:], 1.0)

            # xb slice first (small), then chunked fc load
            xbs_sb = xp.tile([128, KT, MLOC], BF16)
            nc.sync.dma_start(xbs_sb[:], xbs_d[:])
            ft = []
            for ot in range(OT):
                f = fcp.tile([128, C], BF16, tag=f"fc{ot}")
                nc.sync.dma_start(f[:], fc_d[:, ot, :])
                ft.append(f)

            # nb slice: square on ScalarE, column-sum over 32 k-tiles
            x2 = xp.tile([128, KT, MLOC], BF16)
            nc.scalar.square(x2[:], xbs_sb[:])
            psb = psn.tile([1, MLOC], F32)
            for kt in range(KT):
                nc.tensor.matmul(
                    psb[:], ones[:], x2[:, kt, :],
                    start=(kt == 0), stop=(kt == KT - 1),
                )
            st = stage.tile([1, MLOC], F32)
            nc.vector.tensor_copy(st[:], psb[:])
            nc.sync.dma_start(nbsl_d[:], st[:])

            # partial fc column sums; 4 psum banks per half
            wsb = stage.tile([1, C], F32)
            for half in range(2):
                pss = [psw.tile([1, 512], F32, name=f"psw{half}_{i}", tag="psw")
                       for i in range(4)]
                for ot in range(OT):
                    for ci, ps in enumerate(pss):
                        ch = half * 4 + ci
                        nc.tensor.matmul(
                            ps[:], ones[:],
                            ft[ot][:, ch * 512:(ch + 1) * 512],
                            start=(ot == 0), stop=(ot == OT - 1),
                        )
                for ci, ps in enumerate(pss):
                    ch = half * 4 + ci
                    nc.vector.tensor_copy(wsb[:, ch * 512:(ch + 1) * 512], ps[:])
            nc.sync.dma_start(wpart_d[:], wsb[:])

    nc.compile()
    return nc


def _build_phase2():
    """Per-core: 256 rows of scores = (wa+wb)^2 * relu(na+nb-2ab)."""
    nc = _new_nc()
    xasc_d = nc.dram_tensor("xasc", [128, KT, MLOC], FP8, kind="ExternalInput").ap()
    xbr_d = nc.dram_tensor("xbr", [128, KT, CA], FP8, kind="ExternalInput").ap()
    # wa per-partition per m-tile
    pv_d = nc.dram_tensor("pv", [128, 2], F32, kind="ExternalInput").ap()
    # packed free-axis vectors: [0, 0:CA]=wb, [0, CA:2CA]=nb
    fv_d = nc.dram_tensor("fv", [1, 2 * CA], F32, kind="ExternalInput").ap()
    out_d = nc.dram_tensor("scores", [MLOC, CA], F32, kind="ExternalOutput").ap()
    na_dram = nc.dram_tensor("na_tmp", [1, MLOC], F32).ap()

    NJ = CA // 512   # 4 column chunks
    MT = MLOC // 128  # 2 m-tiles
    KG = 2           # k-tiles per xb DMA chunk
    XG = 8           # k-tiles per xa DMA chunk

    with tile.TileContext(nc) as tc:
        with (
            tc.tile_pool(name="xap", bufs=1) as xap,
            tc.tile_pool(name="xbp", bufs=1) as xbp,
            tc.tile_pool(name="small", bufs=1) as small,
            tc.tile_pool(name="w2p", bufs=1) as w2p,
            tc.tile_pool(name="x2p", bufs=2) as x2p,
            tc.tile_pool(name="outp", bufs=2) as outp,
        ):
            # ---- input streams (emission order = DMA issue order) ----
            xac = []
            for g in range(KT // XG):
                x_t = xap.tile([128, XG, MLOC], FP8, tag=f"xa{g}")
                nc.sync.dma_start(x_t[:], xasc_d[:, g * XG:(g + 1) * XG, :])
                xac.append(x_t)
            xbt = []
            for h in range(KT // KG):
                xb_t = xbp.tile([128, KG, CA], FP8, tag=f"xb{h}")
                nc.sync.dma_start(xb_t[:], xbr_d[:, h * KG:(h + 1) * KG, :])
                xbt.append(xb_t)

            quarter = small.tile([128, 1], BF16)
            nc.vector.memset(quarter[:], 0.25)

            # ---- na from (-2a)^2 * 0.25, then DRAM roundtrip to [128,2] ----
            with tc.tile_pool(name="psna", bufs=1, space="PSUM") as psna:
                psa = psna.tile([1, MLOC], F32)
                for g in range(KT // XG):
                    x2 = x2p.tile([128, XG, MLOC], BF16, tag="x2")
                    nc.scalar.square(x2[:], xac[g][:])
                    for s in range(XG):
                        kt = g * XG + s
                        nc.tensor.matmul(
                            psa[:], quarter[:], x2[:, s, :],
                            start=(kt == 0), stop=(kt == KT - 1),
                        )
                nast = small.tile([1, MLOC], F32)
                nc.vector.tensor_copy(nast[:], psa[:])
                nc.sync.dma_start(na_dram[:], nast[:])
            nav = small.tile([128, MT], F32)
            nc.sync.dma_start(
                nav[:],
                bass.AP(tensor=na_dram.tensor, offset=0, ap=[[1, 128], [128, MT]]),
            )

            # ---- main matmul: k-OUTER accumulation, 2 x 4-bank psum tiles ----
            with tc.tile_pool(name="psmm", bufs=2, space="PSUM") as psmm:
                ps = [psmm.tile([128, NJ, 512], F32, name=f"ps{m}", tag="ps")
                      for m in range(MT)]
                for kt in range(KT):
                    h, r = divmod(kt, KG)
                    g, s = divmod(kt, XG)
                    for m in range(MT):
                        for nj in range(NJ):
                            nc.tensor.matmul(
                                ps[m][:, nj, :],
                                xac[g][:, s, m * 128:(m + 1) * 128],
                                xbt[h][:, r, nj * 512:(nj + 1) * 512],
                                start=(kt == 0), stop=(kt == KT - 1),
                            )

                # ---- epilogue vectors (issued late; DMA overlaps MM stream) ----
                pv = small.tile([128, 2], F32)
                nc.sync.dma_start(pv[:], pv_d[:])
                wb_bc = small.tile([128, CA], F32)
                nc.sync.dma_start(wb_bc[:], fv_d[0:1, 0:CA].to_broadcast([128, CA]))
                nb_bc = small.tile([128, CA], F32)
                nc.sync.dma_start(nb_bc[:], fv_d[0:1, CA:2 * CA].to_broadcast([128, CA]))
                w2 = []
                for m in range(MT):
                    w2m = w2p.tile([128, CA], F32, tag=f"w2_{m}")
                    nc.scalar.activation(
                        w2m[:], wb_bc[:],
                        mybir.ActivationFunctionType.Square,
                        bias=pv[:, m:m + 1], scale=1.0,
                    )
                    w2.append(w2m)

                # ---- epilogue: sq in-place in psum, scale, store ----
                for m in range(MT):
                    pflat = ps[m].rearrange("p a b -> p (a b)")
                    nc.vector.scalar_tensor_tensor(
                        pflat, pflat, nav[:, m:m + 1], nb_bc[:],
                        op0=mybir.AluOpType.add, op1=mybir.AluOpType.add,
                    )
                    ot = outp.tile([128, CA], F32, tag="ot")
                    nc.vector.scalar_tensor_tensor(
                        ot[:], pflat, 0.0, w2[m][:],
                        op0=mybir.AluOpType.max, op1=mybir.AluOpType.mult,
                    )
                    nc.sync.dma_start(out_d[m * 128:(m + 1) * 128, :], ot[:])

    nc.compile()
    return nc


def _p_major(a, np_dtype):
    """[n*128, cols] -> [128, n, cols] with tile index in the middle."""
    n = a.shape[0] // 128
    return np.ascontiguousarray(
        a.reshape(n, 128, a.shape[1]).transpose(1, 0, 2).astype(np_dtype)
    )


def _kernel_twolaunch(x, fc_weight, _trace=False):
    x = np.asarray(x, dtype=np.float32)
    fc = np.asarray(fc_weight, dtype=np.float32)

    xf = x.reshape(T, C)
    xa = np.ascontiguousarray(xf[:, 0::2])   # [T, CA]
    xb = np.ascontiguousarray(xf[:, 1::2])
    # deinterleave fc columns: [even | odd] so wpart = [wa_part | wb_part]
    fc_r = np.concatenate([fc[:, 0::2], fc[:, 1::2]], axis=1)

    xb_r = _p_major(xb, NP_FP8)              # [128, KT, CA]
    xa_s2 = -2.0 * xa

    # ---- launch 1 ----
    if "p1" not in _cache:
        _cache["p1"] = _build_phase1()
    nc1 = _cache["p1"]

    in_maps1 = []
    for d in range(D):
        sl = slice(d * MLOC, (d + 1) * MLOC)
        in_maps1.append({
            "fc": _p_major(fc_r[d * OLOC:(d + 1) * OLOC], NP_BF16),
            "xbs": _p_major(xb[:, sl], NP_BF16),
        })
    res1 = run_bass_kernel_spmd(nc1, in_maps1, core_ids=list(range(D)), trace=_trace)
    t1 = res1.exec_time_ns

    wsum = np.sum([res1.results[d]["wpart"][0] for d in range(D)], axis=0,
                  dtype=np.float32)                              # [C] = [wa|wb]
    nb = np.concatenate([res1.results[d]["nbsl"][0] for d in range(D)])
    wa, wb = wsum[:CA], wsum[CA:]

    # ---- launch 2 ----
    if "p2" not in _cache:
        _cache["p2"] = _build_phase2()
    nc2 = _cache["p2"]

    fv = np.concatenate([wb, nb]).reshape(1, 2 * CA).astype(np.float32)
    in_maps2 = []
    for d in range(D):
        sl = slice(d * MLOC, (d + 1) * MLOC)
        in_maps2.append({
            "xasc": _p_major(xa_s2[:, sl], NP_FP8),
            "xbr": xb_r,
            "pv": np.ascontiguousarray(wa[sl].reshape(2, 128).T).astype(np.float32),
            "fv": fv,
        })
    res2 = run_bass_kernel_spmd(nc2, in_maps2, core_ids=list(range(D)), trace=_trace)
    t2 = res2.exec_time_ns

    out = np.concatenate([res2.results[d]["scores"] for d in range(D)], axis=0)
    if _trace:
        kernel.last_times = (t1, t2)
    return out.astype(np.float32)


def _build_merged():
    """Single launch, fc column-sharded per core, odd(wb)/even(wa) halves as
    separate streams: wb half loads first so its AllGather issues early.
    Main matmul fp8; fused fp32 epilogue in PSUM."""
    nc = _new_nc()
    OTT = O // 128    # 96 fc o-tiles
    FG = 16           # o-tiles per fc DMA chunk -> 6 chunks per half
    fcb_d = nc.dram_tensor("fcb", [128, OTT, MLOC], BF16, kind="ExternalInput").ap()
    fca_d = nc.dram_tensor("fca", [128, OTT, MLOC], BF16, kind="ExternalInput").ap()
    xasc_d = nc.dram_tensor("xasc", [128, KT, MLOC], FP8, kind="ExternalInput").ap()
    xbs_d = nc.dram_tensor("xbs", [128, KT, MLOC], FP8, kind="ExternalInput").ap()
    xbr_d = nc.dram_tensor("xbr", [128, KT, CA], FP8, kind="ExternalInput").ap()
    out_d = nc.dram_tensor("scores", [MLOC, CA], F32, kind="ExternalOutput").ap()

    nb_in = nc.dram_tensor("nb_in", [1, MLOC], F32).ap()
    nb_sh = nc.dram_tensor("nb_sh", [D, MLOC], F32, addr_space="Shared").ap()
    wb_in = nc.dram_tensor("wb_in", [1, MLOC], F32).ap()
    wb_sh = nc.dram_tensor("wb_sh", [D, MLOC], F32, addr_space="Shared").ap()
    grp = [list(range(D))]

    NJ = CA // 512
    MT = MLOC // 128
    KG = 2            # k-tiles per xb DMA chunk
    XG = 8            # k-tiles per xa DMA chunk

    import contextlib
    with tile.TileContext(nc) as tc:
        es = contextlib.ExitStack()
        with es, \
             tc.tile_pool(name="xap", bufs=1) as xap, \
             tc.tile_pool(name="xsp", bufs=1) as xsp, \
             tc.tile_pool(name="xbp", bufs=1) as xbp, \
             tc.tile_pool(name="fbp", bufs=2) as fbp, \
             tc.tile_pool(name="fap", bufs=2) as fap, \
             tc.tile_pool(name="small", bufs=1) as small, \
             tc.tile_pool(name="w2p", bufs=1) as w2p, \
             tc.tile_pool(name="x2p", bufs=2) as x2p, \
             tc.tile_pool(name="outp", bufs=2) as outp, \
             tc.tile_pool(name="psm0", bufs=1, space="PSUM") as psm0:
            pse = es.enter_context(tc.tile_pool(name="pse", bufs=1, space="PSUM"))

            # ---- DMA emission: xbs, xasc, fcb (wb half), fca, xbr ----
            xbs_sb = xsp.tile([128, KT, MLOC], FP8)
            nc.sync.dma_start(xbs_sb[:], xbs_d[:])
            xac = []
            for g in range(KT // XG):
                x_t = xap.tile([128, XG, MLOC], FP8, tag=f"xa{g}")
                nc.sync.dma_start(x_t[:], xasc_d[:, g * XG:(g + 1) * XG, :])
                xac.append(x_t)
            fbt = []
            for rnd in range(OTT // FG):
                f = fbp.tile([128, FG, MLOC], BF16, tag="fcb")
                nc.sync.dma_start(f[:], fcb_d[:, rnd * FG:(rnd + 1) * FG, :])
                fbt.append(f)
            fat = []
            for rnd in range(OTT // FG):
                f = fap.tile([128, FG, MLOC], BF16, tag="fca")
                nc.sync.dma_start(f[:], fca_d[:, rnd * FG:(rnd + 1) * FG, :])
                fat.append(f)
            xbt = []
            for bi in range(KT // KG):
                xb_t = xbp.tile([128, KG, CA], FP8, tag=f"xb{bi}")
                nc.sync.dma_start(xb_t[:], xbr_d[:, bi * KG:(bi + 1) * KG, :])
                xbt.append(xb_t)

            ones = small.tile([128, 1], BF16)
            nc.vector.memset(ones[:], 1.0)
            quarter = small.tile([128, 1], BF16)
            nc.vector.memset(quarter[:], 0.25)
            onef = small.tile([1, 1], F32)
            nc.vector.memset(onef[:], 1.0)

            # ---- nb slice (feeds earliest AllGather) ----
            psb = pse.tile([1, MLOC], F32, name="psb", tag="psb")
            for g in range(KT // XG):
                x2b = x2p.tile([128, XG, MLOC], BF16, tag="x2b")
                nc.scalar.square(x2b[:], xbs_sb[:, g * XG:(g + 1) * XG, :])
                for st_ in range(XG):
                    kt = g * XG + st_
                    nc.tensor.matmul(psb[:], ones[:], x2b[:, st_, :],
                                     start=(kt == 0), stop=(kt == KT - 1))
            nbst = small.tile([1, MLOC], F32)
            nc.vector.tensor_copy(nbst[:], psb[:])
            nc.gpsimd.dma_start(nb_in[:], nbst[:])
            nc.gpsimd.collective_compute(
                "AllGather", mybir.AluOpType.bypass, replica_groups=grp,
                ins=[nb_in[:]], outs=[nb_sh[:]])

            # ---- fcb (odd cols): wb_part -> AllGather ASAP ----
            pswb = pse.tile([1, MLOC], F32, name="pswb", tag="bchain")
            for rnd in range(OTT // FG):
                for o in range(FG):
                    ot = rnd * FG + o
                    nc.tensor.matmul(pswb[:], ones[:], fbt[rnd][:, o, :],
                                     start=(ot == 0), stop=(ot == OTT - 1))
            wbst = small.tile([1, MLOC], F32)
            nc.vector.tensor_copy(wbst[:], pswb[:])
            nc.gpsimd.dma_start(wb_in[:], wbst[:])
            nc.gpsimd.collective_compute(
                "AllGather", mybir.AluOpType.bypass, replica_groups=grp,
                ins=[wb_in[:]], outs=[wb_sh[:]])
            # broadcast reads on gpsimd (gated only on the AGs)
            nb_bc = small.tile([128, CA], F32)
            nc.gpsimd.dma_start(nb_bc[:], bass.AP(tensor=nb_sh.tensor, offset=0,
                                                  ap=[[0, 128], [1, CA]]))
            wb_bc = small.tile([128, CA], F32)
            nc.gpsimd.dma_start(wb_bc[:], bass.AP(tensor=wb_sh.tensor, offset=0,
                                                  ap=[[0, 128], [1, CA]]))

            # ---- na local + transpose to [128, MT] via K=1 matmuls ----
            psa = pse.tile([1, MLOC], F32, name="psa", tag="psa")
            for g in range(KT // XG):
                x2 = x2p.tile([128, XG, MLOC], BF16, tag="x2")
                nc.scalar.square(x2[:], xac[g][:])
                for st_ in range(XG):
                    kt = g * XG + st_
                    nc.tensor.matmul(psa[:], quarter[:], x2[:, st_, :],
                                     start=(kt == 0), stop=(kt == KT - 1))
            nast = small.tile([1, MLOC], F32)
            nc.vector.tensor_copy(nast[:], psa[:])
            pst_a = pse.tile([128, MT], F32, name="pst_a", tag="wchain")
            for m in range(MT):
                nc.tensor.matmul(pst_a[:, m:m + 1],
                                 nast[0:1, m * 128:(m + 1) * 128], onef[:],
                                 start=(m == 0), stop=(m == MT - 1),
                                 skip_group_check=True)
            nav = small.tile([128, MT], F32)
            nc.vector.tensor_copy(nav[:], pst_a[:])

            # ---- fca (even cols): wa local ----
            pswa = pse.tile([1, MLOC], F32, name="pswa", tag="wchain")
            for rnd in range(OTT // FG):
                for o in range(FG):
                    ot = rnd * FG + o
                    nc.tensor.matmul(pswa[:], ones[:], fat[rnd][:, o, :],
                                     start=(ot == 0), stop=(ot == OTT - 1))
            wast = small.tile([1, MLOC], F32)
            nc.vector.tensor_copy(wast[:], pswa[:])
            pst_w = pse.tile([128, MT], F32, name="pst_w", tag="wchain")
            for m in range(MT):
                nc.tensor.matmul(pst_w[:, m:m + 1],
                                 wast[0:1, m * 128:(m + 1) * 128], onef[:],
                                 start=(m == 0), stop=(m == MT - 1),
                                 skip_group_check=True)
            wav = small.tile([128, MT], F32)
            nc.vector.tensor_copy(wav[:], pst_w[:])

            # ---- m0 matmuls (chase xbr stream) ----
            ps0 = psm0.tile([128, NJ, 512], F32, name="ps0", tag="ps")
            for kt in range(KT):
                g, s_ = divmod(kt, XG)
                h, r_ = divmod(kt, KG)
                for nj in range(NJ):
                    nc.tensor.matmul(
                        ps0[:, nj, :],
                        xac[g][:, s_, 0:128],
                        xbt[h][:, r_, nj * 512:(nj + 1) * 512],
                        start=(kt == 0), stop=(kt == KT - 1))

            es.close()

            w2 = []
            for m in range(MT):
                w2m = w2p.tile([128, CA], F32, tag=f"w2_{m}")
                nc.scalar.activation(w2m[:], wb_bc[:],
                                     mybir.ActivationFunctionType.Square,
                                     bias=wav[:, m:m + 1], scale=1.0)
                w2.append(w2m)

            with tc.tile_pool(name="psm1", bufs=1, space="PSUM") as psm1:
                ps1 = psm1.tile([128, NJ, 512], F32, name="ps1", tag="ps")
                for kt in range(KT):
                    g, s_ = divmod(kt, XG)
                    h, r_ = divmod(kt, KG)
                    for nj in range(NJ):
                        nc.tensor.matmul(
                            ps1[:, nj, :],
                            xac[g][:, s_, 128:256],
                            xbt[h][:, r_, nj * 512:(nj + 1) * 512],
                            start=(kt == 0), stop=(kt == KT - 1))

                for m, psm in ((0, ps0), (1, ps1)):
                    pflat = psm.rearrange("p a b -> p (a b)")
                    nc.vector.scalar_tensor_tensor(
                        pflat, pflat, nav[:, m:m + 1], nb_bc[:],
                        op0=mybir.AluOpType.add, op1=mybir.AluOpType.add)
                    ot = outp.tile([128, CA], F32, tag="ot")
                    nc.vector.scalar_tensor_tensor(
                        ot[:], pflat, 0.0, w2[m][:],
                        op0=mybir.AluOpType.max, op1=mybir.AluOpType.mult)
                    nc.sync.dma_start(out_d[m * 128:(m + 1) * 128, :], ot[:])

    nc.compile()
    return nc


def kernel_merged(x, fc_weight, _trace=False):
    x = np.asarray(x, dtype=np.float32)
    fc = np.asarray(fc_weight, dtype=np.float32)
    xf = x.reshape(T, C)
    xa = np.ascontiguousarray(xf[:, 0::2])
    xb = np.ascontiguousarray(xf[:, 1::2])
    xb_r = _p_major(xb, NP_FP8)
    xa_s2 = -2.0 * xa

    if "pm" not in _cache:
        _cache["pm"] = _build_merged()
    ncm = _cache["pm"]
    in_maps = []
    for d in range(D):
        sl = slice(d * MLOC, (d + 1) * MLOC)
        lo, hi = 2 * d * MLOC, 2 * (d + 1) * MLOC
        in_maps.append({
            "fcb": _p_major(np.ascontiguousarray(fc[:, lo + 1:hi:2]), NP_BF16),
            "fca": _p_major(np.ascontiguousarray(fc[:, lo:hi:2]), NP_BF16),
            "xasc": _p_major(xa_s2[:, sl], NP_FP8),
            "xbs": _p_major(xb[:, sl], NP_FP8),
            "xbr": xb_r,
        })
    res = run_bass_kernel_spmd(ncm, in_maps, core_ids=list(range(D)), trace=_trace)
    out = np.concatenate([res.results[d]["scores"] for d in range(D)], axis=0)
    if _trace:
        kernel_merged.last_times = (res.exec_time_ns,)
    return out.astype(np.float32)


def kernel(x, fc_weight):
    """Graded entrypoint: full inputs in, full [2048, 2048] scores out."""
    return kernel_merged(x, fc_weight)
